# revision 26
# baseline (speedup 1.0000x reference)
"""Causal self-attention (b=2, t=2048, n_embd=768, n_head=12) on 8 TRN2 cores.

Sharding: core c -> batch b = c // 4, head group g = c % 4 (3 heads per group).
Each core computes, for its batch slice x_b [2048, 768] and its 3 heads:
  qkv slice -> per-head causal attention -> partial projection y_part [2048, 768]
using w_proj rows for those heads.  Host sums the 4 partial outputs per batch.

Structure (vs the v1 baseline, 233786ns -> ~167000ns):
  - x is transposed on the HOST: the DRAM input is xT [768, 2048], so the
    on-core xT [128e, 6, 2048t] loads with plain DMAs (no PE transposes).
  - float32r is bit-identical to f32; the DRAM params are declared f32r so
    DMAs feed the f32r matmuls directly (no staging copies).
  - Scores for heads 0/1 are emitted as ADJACENT K=64 matmuls at row
    tile_positions (0,0)/(64,0); head 2's scores pair across chunks via a
    row-shifted copy of kT2 (q2k2 rows 64:128).  Row-disjoint adjacent
    matmuls run concurrently in the PE array.
  - Softmax 1/S uses DVE reciprocal_approx_fast (via an SBUF bounce - the
    custom DVE ops read garbage from PSUM) + GPSIMD partition_broadcast:
    the ACT engine runs Exp only (1 table load total, vs 25 in v1), and no
    PE broadcast matmuls.
  - Causal diagonal masking via GPSIMD affine_select directly on the expT
    tile (DVE freed up).
  - PSUM->SBUF evacuations split between DVE and ACT (Copy is in every ACT
    table set, so it forces no table reload).

On-core layout (head_dim on partitions):
  xT   [128e, 6, 2048t]  (DMA direct)
  k01  [128, 2048] = [kT_h0 ; kT_h1]   (partitions 0-63 / 64-127)
  q01  [128, 2048] = [qT_h0 ; qT_h1]
  kq2  [128, 2048] = [kT_h2 ; qT_h2]; q2k2 [128, 2048] = [qT_h2 at base 0 ;
  kT_h2 at base 64] via SBUF->SBUF DMA (enables cross-chunk score pairing)
  va   [128k, 16, 3, 65]  v natural + ones column (softmax denominator trick)
  scores sT[k,q] = kT.T@qT per 128k x 512q tile (diagonal tiles trimmed),
  exp on ACT, triangular mask via gpsimd on the 128-wide diagonal block,
  att@v accumulated in PSUM as outT [64d, 512q] with row 64 = denominator.

All matmuls run in float32r (TF32-like, 1 cycle/col at N>=256, ~1e-4 rel err).
"""

import os
import numpy as np
from collections import deque
from contextlib import ExitStack

import concourse.bass as bass
import concourse.mybir as mybir
import concourse.tile as tile
from concourse import bacc
from concourse.bass_utils import run_bass_kernel_spmd

F32 = mybir.dt.float32
F32R = mybir.dt.float32r
AF = mybir.ActivationFunctionType

T = 2048
E = 768
D = 64  # head dim
EC = E // 128  # 6 e-chunks
TC = T // 128  # 16 t-chunks
NS = T // 512  # 4 q-strips
SCALE = 1.0 / 8.0  # 1/sqrt(64)

_CACHED = {}


def build_nc():
    nc = bacc.Bacc("TRN2", target_bir_lowering=False, debug=False)
    xt_d = nc.declare_dram_parameter("xt", [E, T], F32R, isOutput=False)
    wqk_d = nc.declare_dram_parameter("wqk", [E, 384], F32R, isOutput=False)
    wv_d = nc.declare_dram_parameter("wv", [E, 192], F32R, isOutput=False)
    wp_d = nc.declare_dram_parameter("wp", [192, E], F32R, isOutput=False)
    y_d = nc.declare_dram_parameter("y", [T, E], F32, isOutput=True)

    with tile.TileContext(nc) as tc, ExitStack() as ctx:
        singles = ctx.enter_context(tc.tile_pool(name="singles", bufs=1))
        pool_exp = ctx.enter_context(tc.tile_pool(name="exp", bufs=10))
        pool_tmp = ctx.enter_context(tc.tile_pool(name="tmp", bufs=4))
        pool_y = ctx.enter_context(tc.tile_pool(name="yout", bufs=3))
        # PSUM budget: mm 6 banks + acc 2 banks = 8
        ps_main = ctx.enter_context(tc.tile_pool(name="psmain", bufs=6, space="PSUM"))
        ps_acc = ctx.enter_context(tc.tile_pool(name="psacc", bufs=2, space="PSUM"))

        def mm_tile(name):
            return ps_main.tile([128, 512], F32, tag="mm", name=name)

        # ---- weights: f32r is f32 bit-identical, DMA straight in ----
        wqk_r = singles.tile([128, EC, 384], F32R, tag="wqk_r")
        wv_r = singles.tile([128, EC, 192], F32R, tag="wv_r")
        wp01_r = singles.tile([128, E], F32R, tag="wp01_r")
        wp2_r = singles.tile([64, E], F32R, tag="wp2_r")
        nc.sync.dma_start(wqk_r[:], wqk_d.rearrange("(eo p) c -> p eo c", p=128))
        nc.sync.dma_start(wv_r[:], wv_d.rearrange("(eo p) c -> p eo c", p=128))
        nc.sync.dma_start(wp01_r[:], wp_d[0:128, :])
        nc.sync.dma_start(wp2_r[:], wp_d[128:192, :])

        # ---- xT: DMA in strip-granular slabs (strip-0 slabs first) ----
        xT = singles.tile([128, EC, T], F32R, tag="xT")
        for s in range(NS):
            for ec in range(EC):
                nc.sync.dma_start(
                    xT[:, ec, s * 512 : (s + 1) * 512],
                    xt_d[ec * 128 : (ec + 1) * 128, s * 512 : (s + 1) * 512],
                )

        # ---- HAM warmup scratch: dummy matmuls keep the PE busy while the
        # weight/x DMA prologue lands ----
        scratch_f32 = singles.tile([128, 512], F32, tag="scratch_f32")
        nc.vector.memset(scratch_f32, 1.0)
        scratch_r = singles.tile([128, 512], F32R, tag="scratch_r")
        nc.vector.tensor_copy(scratch_r[:], scratch_f32[:])

        # ---- persistent intermediates ----
        k01 = singles.tile([128, T], F32R, tag="k01")
        q01 = singles.tile([128, T], F32R, tag="q01")
        kq2 = singles.tile([128, T], F32R, tag="kq2")
        # q2k2: rows 0:64 = qT_h2 re-homed to base 0; rows 64:128 = kT_h2
        # shifted to base 64 (lets track B pair its scores across chunks)
        q2k2 = singles.tile([128, T], F32R, tag="q2k2")
        va = singles.tile([128, TC, 3, 65], F32R, tag="va")
        u01 = singles.tile([128, T], F32R, tag="u01")
        u2 = singles.tile([64, T], F32R, tag="u2")

        # ones columns of va (denominator accumulator weights)
        ones_col = singles.tile([128, TC * 3], F32, tag="ones_col")
        nc.vector.memset(ones_col, 1.0)
        nc.vector.tensor_copy(
            va[:, :, :, 64], ones_col[:].rearrange("p (t h) -> p t h", h=3)
        )

        # ---------- filler machinery ----------
        soft_q = deque()

        def pop_filler():
            if soft_q:
                soft_q.popleft()()

        def drain_all():
            while soft_q:
                soft_q.popleft()()

        # ---------- emission building blocks ----------
        qkT = [k01, q01, kq2]

        def emit_qk(cc, s):
            pq = mm_tile("pq")
            for ec in range(EC):
                nc.tensor.matmul(
                    pq[:],
                    wqk_r[:, ec, cc * 128 : (cc + 1) * 128],
                    xT[:, ec, s * 512 : (s + 1) * 512],
                    start=(ec == 0),
                    stop=(ec == EC - 1),
                )
            nc.vector.tensor_copy(qkT[cc][:, s * 512 : (s + 1) * 512], pq[:])
            if cc == 2:
                # re-home qT_h2 (partitions 64:128) to base 0, and kT_h2 to
                # base 64 (lets track B pair its scores across chunks)
                nc.sync.dma_start(
                    q2k2[0:64, s * 512 : (s + 1) * 512],
                    kq2[64:128, s * 512 : (s + 1) * 512],
                )
                nc.sync.dma_start(
                    q2k2[64:128, s * 512 : (s + 1) * 512],
                    kq2[0:64, s * 512 : (s + 1) * 512],
                )

        def emit_v(t_i):
            pv = mm_tile("pv")
            for ec in range(EC):
                nc.tensor.matmul(
                    pv[:, 0:192],
                    xT[:, ec, t_i * 128 : (t_i + 1) * 128],
                    wv_r[:, ec, :],
                    start=(ec == 0),
                    stop=(ec == EC - 1),
                )
            nc.vector.tensor_copy(
                va[:, t_i, :, 0:64],
                pv[:, 0:192].rearrange("p (h c) -> p h c", c=64),
            )

        def emit_prep(s):
            for cc in range(3):
                emit_qk(cc, s)
            for t_i in range(4 * s, 4 * s + 4):
                emit_v(t_i)

        def emit_norm(s, h, acc):
            # 1/S via DVE approx (~18 bits), broadcast via gpsimd: the ACT
            # engine stays exp-only and the PE does no broadcast matmuls.
            den_sb = pool_tmp.tile([1, 512], F32, tag="den_sb", name="den_sb")
            nc.vector.tensor_copy(den_sb[:], acc[64:65, :])
            recip = pool_tmp.tile([1, 512], F32, tag="recip", name="recip")
            nc.vector.reciprocal_approx_fast(recip[:], den_sb[:])
            bcs = pool_tmp.tile([64, 512], F32, tag="bcs", name="bcs")
            nc.gpsimd.partition_broadcast(bcs[:], recip[:], channels=64)
            if h == 0:
                nc.vector.tensor_mul(
                    u01[0:64, s * 512 : (s + 1) * 512], acc[0:64, :], bcs[:]
                )
            elif h == 2:
                nc.vector.tensor_mul(
                    u2[0:64, s * 512 : (s + 1) * 512], acc[0:64, :], bcs[:]
                )
            else:
                # head 1 lands at partitions 64:128 of u01 -> shift via DMA
                stage = pool_tmp.tile([64, 512], F32R, tag="stage", name="stage")
                nc.vector.tensor_mul(stage[:], acc[0:64, :], bcs[:])
                nc.sync.dma_start(u01[64:128, s * 512 : (s + 1) * 512], stage[:])

        def emit_proj(t_i, eh, y_sb):
            pp = mm_tile("pp")
            nc.tensor.matmul(
                pp[:, 0:384],
                u01[:, t_i * 128 : (t_i + 1) * 128],
                wp01_r[:, eh * 384 : (eh + 1) * 384],
                start=True,
                stop=False,
            )
            nc.tensor.matmul(
                pp[:, 0:384],
                u2[0:64, t_i * 128 : (t_i + 1) * 128],
                wp2_r[0:64, eh * 384 : (eh + 1) * 384],
                start=False,
                stop=True,
            )
            # alternate evacuation engine: ACT's Copy lives in every table set
            if eh == 0:
                nc.scalar.copy(y_sb[:, 0:384], pp[:, 0:384])
            else:
                nc.vector.tensor_copy(y_sb[:, 384:768], pp[:, 0:384])
                nc.sync.dma_start(y_d[t_i * 128 : (t_i + 1) * 128, :], y_sb[:])

        def queue_proj(s):
            for qc in range(4):
                t_i = s * 4 + qc
                y_sb = pool_y.tile([128, E], F32, tag="y", name="y_sb")
                for eh in range(2):
                    soft_q.append(
                        lambda t_i=t_i, eh=eh, y_sb=y_sb: emit_proj(t_i, eh, y_sb)
                    )

        def emit_scores_one(lhs_t, rhs_t, plo, phi, kc, s, o, exps, key):
            pss = mm_tile("pss")
            nc.tensor.matmul(
                pss[:, o:512],
                lhs_t[plo:phi, kc * 128 : (kc + 1) * 128],
                rhs_t[plo:phi, s * 512 + o : (s + 1) * 512],
                start=True,
                stop=True,
            )
            expT = pool_exp.tile([128, 512], F32R, tag="expT", name="expT")
            nc.scalar.activation(expT[:, o:512], pss[:, o:512], AF.Exp, scale=SCALE)
            if o or kc == 4 * s:
                # diagonal block: zero future positions (query < key)
                nc.gpsimd.affine_select(
                    out=expT[:, o : o + 128],
                    in_=expT[:, o : o + 128],
                    compare_op=mybir.AluOpType.is_ge,
                    fill=0.0,
                    base=0,
                    pattern=[[1, 128]],
                    channel_multiplier=-1,
                )
            exps[key] = expT

        def emit_attv_one(h, acc, kc, s, n, o, exps, key):
            expT = exps.pop(key)
            nc.tensor.matmul(
                acc[0:65, o:512],
                va[:, kc, h, 0:65],
                expT[:, o:512],
                start=(kc == 0),
                stop=(kc == n - 1),
            )

        # ---------- main pipeline ----------
        # a same-bank accumulation chain keeps the PE busy while the
        # prologue DMAs land
        warm_ps = mm_tile("warm_ps")
        NWARM = 20
        for i in range(NWARM):
            nc.tensor.matmul(
                warm_ps[0:64, :],
                scratch_r[0:64, 0:64],
                scratch_r[0:64, :],
                start=(i == 0),
                stop=(i == NWARM - 1),
            )
        emit_prep(0)

        for s in range(NS):
            n = 4 * (s + 1)

            def off(kc, s=s):
                j = kc - 4 * s
                return 0 if j < 0 else j * 128

            exps = {}

            def track_a(s=s, n=n, exps=exps, off=off):
                # ---- track A: heads 0/1, scores paired via row tiles ----
                acc0 = ps_acc.tile([128, 512], F32, tag="acc", name=f"acc{s}h0")
                acc1 = ps_acc.tile([128, 512], F32, tag="acc", name=f"acc{s}h1")
                G = 2  # chunks per group: 4 score tiles in flight
                for g in range(n // G):
                    for kc in range(g * G, (g + 1) * G):
                        o = off(kc)
                        emit_scores_one(k01, q01, 0, 64, kc, s, o, exps, (0, kc))
                        emit_scores_one(k01, q01, 64, 128, kc, s, o, exps, (1, kc))
                    if g > 0:
                        for kc in range((g - 1) * G, g * G):
                            o = off(kc)
                            emit_attv_one(0, acc0, kc, s, n, o, exps, (0, kc))
                            emit_attv_one(1, acc1, kc, s, n, o, exps, (1, kc))
                    pop_filler()
                for kc in range(n - G, n):
                    o = off(kc)
                    emit_attv_one(0, acc0, kc, s, n, o, exps, (0, kc))
                    emit_attv_one(1, acc1, kc, s, n, o, exps, (1, kc))
                emit_norm(s, 0, acc0)
                emit_norm(s, 1, acc1)

            def track_b(s=s, n=n, exps=exps, off=off):
                # ---- track B: head 2, chunk-paired scores via the
                # row-shifted kT2 copy in q2k2 ----
                acc2 = ps_acc.tile([128, 512], F32, tag="acc", name=f"acc{s}h2")
                G2 = 4
                for g in range(n // G2):
                    for kc in range(g * G2, (g + 1) * G2, 2):
                        emit_scores_one(kq2, q2k2, 0, 64, kc, s, off(kc), exps, (2, kc))
                        emit_scores_one(
                            q2k2, kq2, 64, 128, kc + 1, s, off(kc + 1), exps, (2, kc + 1)
                        )
                    if g > 0:
                        for kc in range((g - 1) * G2, g * G2):
                            emit_attv_one(2, acc2, kc, s, n, off(kc), exps, (2, kc))
                    pop_filler()
                    pop_filler()
                for kc in range(n - G2, n):
                    emit_attv_one(2, acc2, kc, s, n, off(kc), exps, (2, kc))
                emit_norm(s, 2, acc2)

            # final strip: run head 2 first so the tail's projection only
            # waits on heads 0/1's (parallel) norm chains
            if s == NS - 1:
                track_b()
                track_a()
            else:
                track_a()
                track_b()

            # next strip's prep runs as one contiguous block (its outputs are
            # read by the very next scores, so it cannot be deferred); this
            # strip's projection is queued as filler for the next strip.
            if s + 1 < NS:
                emit_prep(s + 1)
            queue_proj(s)

        drain_all()

    nc.compile()
    return nc


def _shard_inputs(x, w_qkv, w_proj):
    in_maps = []
    for c in range(8):
        b, g = c // 4, c % 4
        h0 = 3 * g
        q = slice(h0 * D, (h0 + 2) * D)
        k = slice(E + h0 * D, E + (h0 + 2) * D)
        wqk = np.concatenate(
            [
                w_qkv[:, k],  # k_h0 | k_h1
                w_qkv[:, q],  # q_h0 | q_h1
                w_qkv[:, E + (h0 + 2) * D : E + (h0 + 3) * D],  # k_h2
                w_qkv[:, (h0 + 2) * D : (h0 + 3) * D],  # q_h2
            ],
            axis=1,
        )
        wv = w_qkv[:, 2 * E + h0 * D : 2 * E + (h0 + 3) * D]
        wp = w_proj[h0 * D : (h0 + 3) * D, :]
        in_maps.append(
            {
                "xt": np.ascontiguousarray(x[b].T),
                "wqk": np.ascontiguousarray(wqk),
                "wv": np.ascontiguousarray(wv),
                "wp": np.ascontiguousarray(wp),
            }
        )
    return in_maps


def kernel(x, w_qkv, w_proj):
    x = np.asarray(x, dtype=np.float32)
    w_qkv = np.asarray(w_qkv, dtype=np.float32)
    w_proj = np.asarray(w_proj, dtype=np.float32)

    if "nc" not in _CACHED:
        _CACHED["nc"] = build_nc()
    nc = _CACHED["nc"]

    in_maps = _shard_inputs(x, w_qkv, w_proj)
    trace = bool(int(os.environ.get("KERNEL_TRACE", "0")))
    res = run_bass_kernel_spmd(
        nc, in_maps, core_ids=list(range(8)), trace=trace
    )
    _CACHED["last_results"] = res

    y = np.zeros((2, T, E), dtype=np.float32)
    for c in range(8):
        y[c // 4] += res.results[c]["y"]
    return y


# revision 27
# speedup vs baseline: 1.1622x; 1.1622x over previous
"""Causal self-attention (b=2, t=2048, n_embd=768, n_head=12) on 8 TRN2 cores.

Sharding: core c -> batch b = c // 4, head group g = c % 4 (3 heads per group).
Each core computes, for its batch slice x_b [2048, 768] and its 3 heads:
  qkv slice -> per-head causal attention -> partial projection y_part [2048, 768]
using w_proj rows for those heads.  Host sums the 4 partial outputs per batch.

Structure (vs the v1 baseline, 233786ns -> ~167000ns):
  - x is transposed on the HOST: the DRAM input is xT [768, 2048], so the
    on-core xT [128e, 6, 2048t] loads with plain DMAs (no PE transposes).
  - float32r is bit-identical to f32; the DRAM params are declared f32r so
    DMAs feed the f32r matmuls directly (no staging copies).
  - Scores for heads 0/1 are emitted as ADJACENT K=64 matmuls at row
    tile_positions (0,0)/(64,0); head 2's scores pair across chunks via a
    row-shifted copy of kT2 (q2k2 rows 64:128).  Row-disjoint adjacent
    matmuls run concurrently in the PE array.
  - Softmax 1/S uses DVE reciprocal_approx_fast (via an SBUF bounce - the
    custom DVE ops read garbage from PSUM) + GPSIMD partition_broadcast:
    the ACT engine runs Exp only (1 table load total, vs 25 in v1), and no
    PE broadcast matmuls.
  - Causal diagonal masking via GPSIMD affine_select directly on the expT
    tile (DVE freed up).
  - PSUM->SBUF evacuations split between DVE and ACT (Copy is in every ACT
    table set, so it forces no table reload).

On-core layout (head_dim on partitions):
  xT   [128e, 6, 2048t]  (DMA direct)
  k01  [128, 2048] = [kT_h0 ; kT_h1]   (partitions 0-63 / 64-127)
  q01  [128, 2048] = [qT_h0 ; qT_h1]
  kq2  [128, 2048] = [kT_h2 ; qT_h2]; q2k2 [128, 2048] = [qT_h2 at base 0 ;
  kT_h2 at base 64] via SBUF->SBUF DMA (enables cross-chunk score pairing)
  va   [128k, 16, 3, 65]  v natural + ones column (softmax denominator trick)
  scores sT[k,q] = kT.T@qT per 128k x 512q tile (diagonal tiles trimmed),
  exp on ACT, triangular mask via gpsimd on the 128-wide diagonal block,
  att@v accumulated in PSUM as outT [64d, 512q] with row 64 = denominator.

All matmuls run in float32r (TF32-like, 1 cycle/col at N>=256, ~1e-4 rel err).
"""

import os
import numpy as np
from collections import deque
from contextlib import ExitStack

import concourse.bass as bass
import concourse.mybir as mybir
import concourse.tile as tile
from concourse import bacc
from concourse.bass_utils import run_bass_kernel_spmd

F32 = mybir.dt.float32
F32R = mybir.dt.float32r
AF = mybir.ActivationFunctionType

T = 2048
E = 768
D = 64  # head dim
EC = E // 128  # 6 e-chunks
TC = T // 128  # 16 t-chunks
NS = T // 512  # 4 q-strips
SCALE = 1.0 / 8.0  # 1/sqrt(64)

_CACHED = {}


def build_nc():
    nc = bacc.Bacc("TRN2", target_bir_lowering=False, debug=False)
    xt_d = nc.declare_dram_parameter("xt", [E, T], F32R, isOutput=False)
    wqk_d = nc.declare_dram_parameter("wqk", [E, 384], F32R, isOutput=False)
    wv_d = nc.declare_dram_parameter("wv", [E, 192], F32R, isOutput=False)
    wp_d = nc.declare_dram_parameter("wp", [192, E], F32R, isOutput=False)
    y_d = nc.declare_dram_parameter("y", [T, E], F32, isOutput=True)

    with tile.TileContext(nc) as tc, ExitStack() as ctx:
        singles = ctx.enter_context(tc.tile_pool(name="singles", bufs=1))
        pool_exp = ctx.enter_context(tc.tile_pool(name="exp", bufs=10))
        pool_tmp = ctx.enter_context(tc.tile_pool(name="tmp", bufs=4))
        pool_y = ctx.enter_context(tc.tile_pool(name="yout", bufs=3))
        # PSUM budget: mm 6 banks + acc 2 banks = 8
        ps_main = ctx.enter_context(tc.tile_pool(name="psmain", bufs=6, space="PSUM"))
        ps_acc = ctx.enter_context(tc.tile_pool(name="psacc", bufs=2, space="PSUM"))

        def mm_tile(name):
            return ps_main.tile([128, 512], F32, tag="mm", name=name)

        # ---- weights: f32r is f32 bit-identical, DMA straight in ----
        wqk_r = singles.tile([128, EC, 384], F32R, tag="wqk_r")
        wv_r = singles.tile([128, EC, 192], F32R, tag="wv_r")
        wp01_r = singles.tile([128, E], F32R, tag="wp01_r")
        wp2_r = singles.tile([64, E], F32R, tag="wp2_r")
        nc.sync.dma_start(wqk_r[:], wqk_d.rearrange("(eo p) c -> p eo c", p=128))
        nc.sync.dma_start(wv_r[:], wv_d.rearrange("(eo p) c -> p eo c", p=128))
        nc.sync.dma_start(wp01_r[:], wp_d[0:128, :])
        nc.sync.dma_start(wp2_r[:], wp_d[128:192, :])

        # ---- xT: DMA in strip-granular slabs (strip-0 slabs first) ----
        xT = singles.tile([128, EC, T], F32R, tag="xT")
        for s in range(NS):
            for ec in range(EC):
                nc.sync.dma_start(
                    xT[:, ec, s * 512 : (s + 1) * 512],
                    xt_d[ec * 128 : (ec + 1) * 128, s * 512 : (s + 1) * 512],
                )

        # ---- HAM warmup scratch: dummy matmuls keep the PE busy while the
        # weight/x DMA prologue lands ----
        scratch_f32 = singles.tile([128, 512], F32, tag="scratch_f32")
        nc.vector.memset(scratch_f32, 1.0)
        scratch_r = singles.tile([128, 512], F32R, tag="scratch_r")
        nc.vector.tensor_copy(scratch_r[:], scratch_f32[:])

        # ---- persistent intermediates ----
        k01 = singles.tile([128, T], F32R, tag="k01")
        q01 = singles.tile([128, T], F32R, tag="q01")
        kq2 = singles.tile([128, T], F32R, tag="kq2")
        # q2k2: rows 0:64 = qT_h2 re-homed to base 0; rows 64:128 = kT_h2
        # shifted to base 64 (lets track B pair its scores across chunks)
        q2k2 = singles.tile([128, T], F32R, tag="q2k2")
        va = singles.tile([128, TC, 3, 65], F32R, tag="va")
        u01 = singles.tile([128, T], F32R, tag="u01")
        u2 = singles.tile([64, T], F32R, tag="u2")

        # ones columns of va (denominator accumulator weights)
        ones_col = singles.tile([128, TC * 3], F32, tag="ones_col")
        nc.vector.memset(ones_col, 1.0)
        nc.vector.tensor_copy(
            va[:, :, :, 64], ones_col[:].rearrange("p (t h) -> p t h", h=3)
        )

        # ---------- filler machinery ----------
        soft_q = deque()

        def pop_filler():
            if soft_q:
                soft_q.popleft()()

        def drain_all():
            while soft_q:
                soft_q.popleft()()

        # ---------- emission building blocks ----------
        qkT = [k01, q01, kq2]

        def emit_qk(cc, s):
            pq = mm_tile("pq")
            for ec in range(EC):
                nc.tensor.matmul(
                    pq[:],
                    wqk_r[:, ec, cc * 128 : (cc + 1) * 128],
                    xT[:, ec, s * 512 : (s + 1) * 512],
                    start=(ec == 0),
                    stop=(ec == EC - 1),
                )
            nc.vector.tensor_copy(qkT[cc][:, s * 512 : (s + 1) * 512], pq[:])
            if cc == 2:
                # re-home qT_h2 (partitions 64:128) to base 0, and kT_h2 to
                # base 64 (lets track B pair its scores across chunks)
                nc.sync.dma_start(
                    q2k2[0:64, s * 512 : (s + 1) * 512],
                    kq2[64:128, s * 512 : (s + 1) * 512],
                )
                nc.sync.dma_start(
                    q2k2[64:128, s * 512 : (s + 1) * 512],
                    kq2[0:64, s * 512 : (s + 1) * 512],
                )

        def emit_v(t_i):
            pv = mm_tile("pv")
            for ec in range(EC):
                nc.tensor.matmul(
                    pv[:, 0:192],
                    xT[:, ec, t_i * 128 : (t_i + 1) * 128],
                    wv_r[:, ec, :],
                    start=(ec == 0),
                    stop=(ec == EC - 1),
                )
            nc.vector.tensor_copy(
                va[:, t_i, :, 0:64],
                pv[:, 0:192].rearrange("p (h c) -> p h c", c=64),
            )

        def emit_prep(s):
            for cc in range(3):
                emit_qk(cc, s)
            for t_i in range(4 * s, 4 * s + 4):
                emit_v(t_i)

        def emit_norm(s, h, acc):
            # 1/S via DVE approx (~18 bits), broadcast via gpsimd: the ACT
            # engine stays exp-only and the PE does no broadcast matmuls.
            den_sb = pool_tmp.tile([1, 512], F32, tag="den_sb", name="den_sb")
            nc.vector.tensor_copy(den_sb[:], acc[64:65, :])
            recip = pool_tmp.tile([1, 512], F32, tag="recip", name="recip")
            nc.vector.reciprocal_approx_fast(recip[:], den_sb[:])
            bcs = pool_tmp.tile([64, 512], F32, tag="bcs", name="bcs")
            nc.gpsimd.partition_broadcast(bcs[:], recip[:], channels=64)
            if h == 0:
                nc.vector.tensor_mul(
                    u01[0:64, s * 512 : (s + 1) * 512], acc[0:64, :], bcs[:]
                )
            elif h == 2:
                nc.vector.tensor_mul(
                    u2[0:64, s * 512 : (s + 1) * 512], acc[0:64, :], bcs[:]
                )
            else:
                # head 1 lands at partitions 64:128 of u01 -> shift via DMA
                stage = pool_tmp.tile([64, 512], F32R, tag="stage", name="stage")
                nc.vector.tensor_mul(stage[:], acc[0:64, :], bcs[:])
                nc.sync.dma_start(u01[64:128, s * 512 : (s + 1) * 512], stage[:])

        def emit_proj(t_i, eh, y_sb):
            pp = mm_tile("pp")
            nc.tensor.matmul(
                pp[:, 0:384],
                u01[:, t_i * 128 : (t_i + 1) * 128],
                wp01_r[:, eh * 384 : (eh + 1) * 384],
                start=True,
                stop=False,
            )
            nc.tensor.matmul(
                pp[:, 0:384],
                u2[0:64, t_i * 128 : (t_i + 1) * 128],
                wp2_r[0:64, eh * 384 : (eh + 1) * 384],
                start=False,
                stop=True,
            )
            # alternate evacuation engine: ACT's Copy lives in every table set
            if eh == 0:
                nc.scalar.copy(y_sb[:, 0:384], pp[:, 0:384])
            else:
                nc.vector.tensor_copy(y_sb[:, 384:768], pp[:, 0:384])
                nc.sync.dma_start(y_d[t_i * 128 : (t_i + 1) * 128, :], y_sb[:])

        def queue_proj(s):
            for qc in range(4):
                t_i = s * 4 + qc
                y_sb = pool_y.tile([128, E], F32, tag="y", name="y_sb")
                for eh in range(2):
                    soft_q.append(
                        lambda t_i=t_i, eh=eh, y_sb=y_sb: emit_proj(t_i, eh, y_sb)
                    )

        def emit_scores_one(lhs_t, rhs_t, plo, phi, kc, s, o, exps, key):
            pss = mm_tile("pss")
            nc.tensor.matmul(
                pss[:, o:512],
                lhs_t[plo:phi, kc * 128 : (kc + 1) * 128],
                rhs_t[plo:phi, s * 512 + o : (s + 1) * 512],
                start=True,
                stop=True,
            )
            expT = pool_exp.tile([128, 512], F32R, tag="expT", name="expT")
            nc.scalar.activation(expT[:, o:512], pss[:, o:512], AF.Exp, scale=SCALE)
            if o or kc == 4 * s:
                # diagonal block: zero future positions (query < key)
                nc.gpsimd.affine_select(
                    out=expT[:, o : o + 128],
                    in_=expT[:, o : o + 128],
                    compare_op=mybir.AluOpType.is_ge,
                    fill=0.0,
                    base=0,
                    pattern=[[1, 128]],
                    channel_multiplier=-1,
                )
            exps[key] = expT

        def emit_attv_one(h, acc, kc, s, n, o, exps, key):
            expT = exps.pop(key)
            nc.tensor.matmul(
                acc[0:65, o:512],
                va[:, kc, h, 0:65],
                expT[:, o:512],
                start=(kc == 0),
                stop=(kc == n - 1),
            )

        # ---------- main pipeline ----------
        # a same-bank accumulation chain keeps the PE busy while the
        # prologue DMAs land
        warm_ps = mm_tile("warm_ps")
        NWARM = 30
        for i in range(NWARM):
            nc.tensor.matmul(
                warm_ps[0:64, :],
                scratch_r[0:64, 0:64],
                scratch_r[0:64, :],
                start=(i == 0),
                stop=(i == NWARM - 1),
            )
        emit_prep(0)

        for s in range(NS):
            n = 4 * (s + 1)

            def off(kc, s=s):
                j = kc - 4 * s
                return 0 if j < 0 else j * 128

            # ---- track A: heads 0 and 1, scores paired via row tiles ----
            acc0 = ps_acc.tile([128, 512], F32, tag="acc", name=f"acc{s}h0")
            acc1 = ps_acc.tile([128, 512], F32, tag="acc", name=f"acc{s}h1")
            exps = {}
            G = 2  # chunks per group: 4 score tiles in flight from ps_main
            ngroups = n // G
            for g in range(ngroups):
                for kc in range(g * G, (g + 1) * G):
                    o = off(kc)
                    emit_scores_one(k01, q01, 0, 64, kc, s, o, exps, (0, kc))
                    emit_scores_one(k01, q01, 64, 128, kc, s, o, exps, (1, kc))
                if g > 0:
                    for kc in range((g - 1) * G, g * G):
                        o = off(kc)
                        emit_attv_one(0, acc0, kc, s, n, o, exps, (0, kc))
                        emit_attv_one(1, acc1, kc, s, n, o, exps, (1, kc))
                pop_filler()
            for kc in range(n - G, n):
                o = off(kc)
                emit_attv_one(0, acc0, kc, s, n, o, exps, (0, kc))
                emit_attv_one(1, acc1, kc, s, n, o, exps, (1, kc))
            emit_norm(s, 0, acc0)
            emit_norm(s, 1, acc1)

            # ---- track B: head 2, chunk-paired scores via the row-shifted
            # kT2 copy in q2k2: chunk kc at rows 0:64, kc+1 at rows 64:128 ----
            acc2 = ps_acc.tile([128, 512], F32, tag="acc", name=f"acc{s}h2")
            G2 = 4
            for g in range(n // G2):
                for kc in range(g * G2, (g + 1) * G2, 2):
                    emit_scores_one(kq2, q2k2, 0, 64, kc, s, off(kc), exps, (2, kc))
                    emit_scores_one(
                        q2k2, kq2, 64, 128, kc + 1, s, off(kc + 1), exps, (2, kc + 1)
                    )
                if g > 0:
                    for kc in range((g - 1) * G2, g * G2):
                        emit_attv_one(2, acc2, kc, s, n, off(kc), exps, (2, kc))
                pop_filler()
                pop_filler()
            for kc in range(n - G2, n):
                emit_attv_one(2, acc2, kc, s, n, off(kc), exps, (2, kc))
            emit_norm(s, 2, acc2)

            # next strip's prep runs as one contiguous block (its outputs are
            # read by the very next scores, so it cannot be deferred); this
            # strip's projection is queued as filler for the next strip.
            if s + 1 < NS:
                emit_prep(s + 1)
            queue_proj(s)

        drain_all()

    nc.compile()
    return nc


def _shard_inputs(x, w_qkv, w_proj):
    in_maps = []
    for c in range(8):
        b, g = c // 4, c % 4
        h0 = 3 * g
        q = slice(h0 * D, (h0 + 2) * D)
        k = slice(E + h0 * D, E + (h0 + 2) * D)
        wqk = np.concatenate(
            [
                w_qkv[:, k],  # k_h0 | k_h1
                w_qkv[:, q],  # q_h0 | q_h1
                w_qkv[:, E + (h0 + 2) * D : E + (h0 + 3) * D],  # k_h2
                w_qkv[:, (h0 + 2) * D : (h0 + 3) * D],  # q_h2
            ],
            axis=1,
        )
        wv = w_qkv[:, 2 * E + h0 * D : 2 * E + (h0 + 3) * D]
        wp = w_proj[h0 * D : (h0 + 3) * D, :]
        in_maps.append(
            {
                "xt": np.ascontiguousarray(x[b].T),
                "wqk": np.ascontiguousarray(wqk),
                "wv": np.ascontiguousarray(wv),
                "wp": np.ascontiguousarray(wp),
            }
        )
    return in_maps


def kernel(x, w_qkv, w_proj):
    x = np.asarray(x, dtype=np.float32)
    w_qkv = np.asarray(w_qkv, dtype=np.float32)
    w_proj = np.asarray(w_proj, dtype=np.float32)

    if "nc" not in _CACHED:
        _CACHED["nc"] = build_nc()
    nc = _CACHED["nc"]

    in_maps = _shard_inputs(x, w_qkv, w_proj)
    trace = bool(int(os.environ.get("KERNEL_TRACE", "0")))
    res = run_bass_kernel_spmd(
        nc, in_maps, core_ids=list(range(8)), trace=trace
    )
    _CACHED["last_results"] = res

    y = np.zeros((2, T, E), dtype=np.float32)
    for c in range(8):
        y[c // 4] += res.results[c]["y"]
    return y


# revision 28
# speedup vs baseline: 1.2267x; 1.0555x over previous
"""Causal self-attention (b=2, t=2048, n_embd=768, n_head=12) on 8 TRN2 cores.

Sharding: core c -> batch b = c // 4, head group g = c % 4 (3 heads per group).
Each core computes, for its batch slice x_b [2048, 768] and its 3 heads:
  qkv slice -> per-head causal attention -> partial projection y_part [2048, 768]
using w_proj rows for those heads.  Host sums the 4 partial outputs per batch.

Structure (vs the v1 baseline, 233786ns -> ~167000ns):
  - x is transposed on the HOST: the DRAM input is xT [768, 2048], so the
    on-core xT [128e, 6, 2048t] loads with plain DMAs (no PE transposes).
  - float32r is bit-identical to f32; the DRAM params are declared f32r so
    DMAs feed the f32r matmuls directly (no staging copies).
  - Scores for heads 0/1 are emitted as ADJACENT K=64 matmuls at row
    tile_positions (0,0)/(64,0); head 2's scores pair across chunks via a
    row-shifted copy of kT2 (q2k2 rows 64:128).  Row-disjoint adjacent
    matmuls run concurrently in the PE array.
  - Softmax 1/S uses DVE reciprocal_approx_fast (via an SBUF bounce - the
    custom DVE ops read garbage from PSUM) + GPSIMD partition_broadcast:
    the ACT engine runs Exp only (1 table load total, vs 25 in v1), and no
    PE broadcast matmuls.
  - Causal diagonal masking via GPSIMD affine_select directly on the expT
    tile (DVE freed up).
  - PSUM->SBUF evacuations split between DVE and ACT (Copy is in every ACT
    table set, so it forces no table reload).

On-core layout (head_dim on partitions):
  xT   [128e, 6, 2048t]  (DMA direct)
  k01  [128, 2048] = [kT_h0 ; kT_h1]   (partitions 0-63 / 64-127)
  q01  [128, 2048] = [qT_h0 ; qT_h1]
  kq2  [128, 2048] = [kT_h2 ; qT_h2]; q2k2 [128, 2048] = [qT_h2 at base 0 ;
  kT_h2 at base 64] via SBUF->SBUF DMA (enables cross-chunk score pairing)
  va   [128k, 16, 3, 65]  v natural + ones column (softmax denominator trick)
  scores sT[k,q] = kT.T@qT per 128k x 512q tile (diagonal tiles trimmed),
  exp on ACT, triangular mask via gpsimd on the 128-wide diagonal block,
  att@v accumulated in PSUM as outT [64d, 512q] with row 64 = denominator.

All matmuls run in float32r (TF32-like, 1 cycle/col at N>=256, ~1e-4 rel err).
"""

import os
import numpy as np
from collections import deque
from contextlib import ExitStack

import concourse.bass as bass
import concourse.mybir as mybir
import concourse.tile as tile
from concourse import bacc
from concourse.bass_utils import run_bass_kernel_spmd

F32 = mybir.dt.float32
F32R = mybir.dt.float32r
AF = mybir.ActivationFunctionType

T = 2048
E = 768
D = 64  # head dim
EC = E // 128  # 6 e-chunks
TC = T // 128  # 16 t-chunks
NS = T // 512  # 4 q-strips
SCALE = 1.0 / 8.0  # 1/sqrt(64)

_CACHED = {}


def build_nc():
    nc = bacc.Bacc("TRN2", target_bir_lowering=False, debug=False)
    xt_d = nc.declare_dram_parameter("xt", [E, T], F32R, isOutput=False)
    wqk_d = nc.declare_dram_parameter("wqk", [E, 384], F32R, isOutput=False)
    wv_d = nc.declare_dram_parameter("wv", [E, 192], F32R, isOutput=False)
    wp_d = nc.declare_dram_parameter("wp", [192, E], F32R, isOutput=False)
    y_d = nc.declare_dram_parameter("y", [T, E], F32, isOutput=True)

    with tile.TileContext(nc) as tc, ExitStack() as ctx:
        singles = ctx.enter_context(tc.tile_pool(name="singles", bufs=1))
        pool_exp = ctx.enter_context(tc.tile_pool(name="exp", bufs=10))
        pool_tmp = ctx.enter_context(tc.tile_pool(name="tmp", bufs=4))
        pool_y = ctx.enter_context(tc.tile_pool(name="yout", bufs=3))
        # PSUM budget: mm 6 banks + acc 2 banks = 8
        ps_main = ctx.enter_context(tc.tile_pool(name="psmain", bufs=6, space="PSUM"))
        ps_acc = ctx.enter_context(tc.tile_pool(name="psacc", bufs=2, space="PSUM"))

        def mm_tile(name):
            return ps_main.tile([128, 512], F32, tag="mm", name=name)

        # ---- weights: f32r is f32 bit-identical, DMA straight in ----
        wqk_r = singles.tile([128, EC, 384], F32R, tag="wqk_r")
        wv_r = singles.tile([128, EC, 192], F32R, tag="wv_r")
        wp01_r = singles.tile([128, E], F32R, tag="wp01_r")
        wp2_r = singles.tile([64, E], F32R, tag="wp2_r")
        nc.sync.dma_start(wqk_r[:], wqk_d.rearrange("(eo p) c -> p eo c", p=128))
        nc.sync.dma_start(wv_r[:], wv_d.rearrange("(eo p) c -> p eo c", p=128))
        nc.sync.dma_start(wp01_r[:], wp_d[0:128, :])
        nc.sync.dma_start(wp2_r[:], wp_d[128:192, :])

        # ---- xT: DMA in strip-granular slabs (strip-0 slabs first) ----
        xT = singles.tile([128, EC, T], F32R, tag="xT")
        for s in range(NS):
            for ec in range(EC):
                nc.sync.dma_start(
                    xT[:, ec, s * 512 : (s + 1) * 512],
                    xt_d[ec * 128 : (ec + 1) * 128, s * 512 : (s + 1) * 512],
                )

        # ---- HAM warmup scratch: dummy matmuls keep the PE busy while the
        # weight/x DMA prologue lands ----
        scratch_f32 = singles.tile([128, 512], F32, tag="scratch_f32")
        nc.vector.memset(scratch_f32, 1.0)
        scratch_r = singles.tile([128, 512], F32R, tag="scratch_r")
        nc.vector.tensor_copy(scratch_r[:], scratch_f32[:])

        # ---- persistent intermediates ----
        k01 = singles.tile([128, T], F32R, tag="k01")
        q01 = singles.tile([128, T], F32R, tag="q01")
        kq2 = singles.tile([128, T], F32R, tag="kq2")
        # q2k2: rows 0:64 = qT_h2 re-homed to base 0; rows 64:128 = kT_h2
        # shifted to base 64 (lets track B pair its scores across chunks)
        q2k2 = singles.tile([128, T], F32R, tag="q2k2")
        va = singles.tile([128, TC, 3, 65], F32R, tag="va")
        u01 = singles.tile([128, T], F32R, tag="u01")
        u2 = singles.tile([64, T], F32R, tag="u2")

        # ones columns of va (denominator accumulator weights)
        ones_col = singles.tile([128, TC * 3], F32, tag="ones_col")
        nc.vector.memset(ones_col, 1.0)
        nc.vector.tensor_copy(
            va[:, :, :, 64], ones_col[:].rearrange("p (t h) -> p t h", h=3)
        )

        # ---------- filler machinery ----------
        soft_q = deque()

        def pop_filler():
            if soft_q:
                soft_q.popleft()()

        def drain_all():
            while soft_q:
                soft_q.popleft()()

        # ---------- emission building blocks ----------
        qkT = [k01, q01, kq2]

        def emit_qk(cc, s):
            pq = mm_tile("pq")
            for ec in range(EC):
                nc.tensor.matmul(
                    pq[:],
                    wqk_r[:, ec, cc * 128 : (cc + 1) * 128],
                    xT[:, ec, s * 512 : (s + 1) * 512],
                    start=(ec == 0),
                    stop=(ec == EC - 1),
                )
            nc.vector.tensor_copy(qkT[cc][:, s * 512 : (s + 1) * 512], pq[:])
            if cc == 2:
                # re-home qT_h2 (partitions 64:128) to base 0, and kT_h2 to
                # base 64 (lets track B pair its scores across chunks)
                nc.sync.dma_start(
                    q2k2[0:64, s * 512 : (s + 1) * 512],
                    kq2[64:128, s * 512 : (s + 1) * 512],
                )
                nc.sync.dma_start(
                    q2k2[64:128, s * 512 : (s + 1) * 512],
                    kq2[0:64, s * 512 : (s + 1) * 512],
                )

        def emit_v(t_i):
            pv = mm_tile("pv")
            for ec in range(EC):
                nc.tensor.matmul(
                    pv[:, 0:192],
                    xT[:, ec, t_i * 128 : (t_i + 1) * 128],
                    wv_r[:, ec, :],
                    start=(ec == 0),
                    stop=(ec == EC - 1),
                )
            nc.vector.tensor_copy(
                va[:, t_i, :, 0:64],
                pv[:, 0:192].rearrange("p (h c) -> p h c", c=64),
            )

        def emit_prep(s):
            for cc in range(3):
                emit_qk(cc, s)
            for t_i in range(4 * s, 4 * s + 4):
                emit_v(t_i)

        def emit_norm(s, h, acc):
            # 1/S via DVE approx (~18 bits), broadcast via gpsimd: the ACT
            # engine stays exp-only and the PE does no broadcast matmuls.
            den_sb = pool_tmp.tile([1, 512], F32, tag="den_sb", name="den_sb")
            nc.vector.tensor_copy(den_sb[:], acc[64:65, :])
            recip = pool_tmp.tile([1, 512], F32, tag="recip", name="recip")
            nc.vector.reciprocal_approx_fast(recip[:], den_sb[:])
            bcs = pool_tmp.tile([64, 512], F32, tag="bcs", name="bcs")
            nc.gpsimd.partition_broadcast(bcs[:], recip[:], channels=64)
            if h == 0:
                nc.vector.tensor_mul(
                    u01[0:64, s * 512 : (s + 1) * 512], acc[0:64, :], bcs[:]
                )
            elif h == 2:
                nc.vector.tensor_mul(
                    u2[0:64, s * 512 : (s + 1) * 512], acc[0:64, :], bcs[:]
                )
            else:
                # head 1 lands at partitions 64:128 of u01 -> shift via DMA
                stage = pool_tmp.tile([64, 512], F32R, tag="stage", name="stage")
                nc.vector.tensor_mul(stage[:], acc[0:64, :], bcs[:])
                nc.sync.dma_start(u01[64:128, s * 512 : (s + 1) * 512], stage[:])

        def emit_proj(t_i, eh, y_sb):
            pp = mm_tile("pp")
            nc.tensor.matmul(
                pp[:, 0:384],
                u01[:, t_i * 128 : (t_i + 1) * 128],
                wp01_r[:, eh * 384 : (eh + 1) * 384],
                start=True,
                stop=False,
            )
            nc.tensor.matmul(
                pp[:, 0:384],
                u2[0:64, t_i * 128 : (t_i + 1) * 128],
                wp2_r[0:64, eh * 384 : (eh + 1) * 384],
                start=False,
                stop=True,
            )
            # alternate evacuation engine: ACT's Copy lives in every table set
            if eh == 0:
                nc.scalar.copy(y_sb[:, 0:384], pp[:, 0:384])
            else:
                nc.vector.tensor_copy(y_sb[:, 384:768], pp[:, 0:384])
                nc.sync.dma_start(y_d[t_i * 128 : (t_i + 1) * 128, :], y_sb[:])

        def queue_proj(s):
            for qc in range(4):
                t_i = s * 4 + qc
                y_sb = pool_y.tile([128, E], F32, tag="y", name="y_sb")
                for eh in range(2):
                    soft_q.append(
                        lambda t_i=t_i, eh=eh, y_sb=y_sb: emit_proj(t_i, eh, y_sb)
                    )

        def emit_scores_one(lhs_t, rhs_t, plo, phi, kc, s, o, exps, key):
            pss = mm_tile("pss")
            nc.tensor.matmul(
                pss[:, o:512],
                lhs_t[plo:phi, kc * 128 : (kc + 1) * 128],
                rhs_t[plo:phi, s * 512 + o : (s + 1) * 512],
                start=True,
                stop=True,
            )
            expT = pool_exp.tile([128, 512], F32R, tag="expT", name="expT")
            nc.scalar.activation(expT[:, o:512], pss[:, o:512], AF.Exp, scale=SCALE)
            if o or kc == 4 * s:
                # diagonal block: zero future positions (query < key)
                nc.gpsimd.affine_select(
                    out=expT[:, o : o + 128],
                    in_=expT[:, o : o + 128],
                    compare_op=mybir.AluOpType.is_ge,
                    fill=0.0,
                    base=0,
                    pattern=[[1, 128]],
                    channel_multiplier=-1,
                )
            exps[key] = expT

        def emit_attv_one(h, acc, kc, s, n, o, exps, key):
            expT = exps.pop(key)
            nc.tensor.matmul(
                acc[0:65, o:512],
                va[:, kc, h, 0:65],
                expT[:, o:512],
                start=(kc == 0),
                stop=(kc == n - 1),
            )

        # ---------- main pipeline ----------
        # a same-bank accumulation chain keeps the PE busy while the
        # prologue DMAs land
        warm_ps = mm_tile("warm_ps")
        NWARM = 30
        for i in range(NWARM):
            nc.tensor.matmul(
                warm_ps[0:64, :],
                scratch_r[:, 0:64],
                scratch_r[:, :],
                start=(i == 0),
                stop=(i == NWARM - 1),
            )
        emit_prep(0)

        for s in range(NS):
            n = 4 * (s + 1)

            def off(kc, s=s):
                j = kc - 4 * s
                return 0 if j < 0 else j * 128

            # ---- track A: heads 0 and 1, scores paired via row tiles ----
            acc0 = ps_acc.tile([128, 512], F32, tag="acc", name=f"acc{s}h0")
            acc1 = ps_acc.tile([128, 512], F32, tag="acc", name=f"acc{s}h1")
            exps = {}
            G = 2  # chunks per group: 4 score tiles in flight from ps_main
            ngroups = n // G
            for g in range(ngroups):
                for kc in range(g * G, (g + 1) * G):
                    o = off(kc)
                    emit_scores_one(k01, q01, 0, 64, kc, s, o, exps, (0, kc))
                    emit_scores_one(k01, q01, 64, 128, kc, s, o, exps, (1, kc))
                if g > 0:
                    for kc in range((g - 1) * G, g * G):
                        o = off(kc)
                        emit_attv_one(0, acc0, kc, s, n, o, exps, (0, kc))
                        emit_attv_one(1, acc1, kc, s, n, o, exps, (1, kc))
                pop_filler()
            for kc in range(n - G, n):
                o = off(kc)
                emit_attv_one(0, acc0, kc, s, n, o, exps, (0, kc))
                emit_attv_one(1, acc1, kc, s, n, o, exps, (1, kc))
            emit_norm(s, 0, acc0)
            emit_norm(s, 1, acc1)

            # ---- track B: head 2, chunk-paired scores via the row-shifted
            # kT2 copy in q2k2: chunk kc at rows 0:64, kc+1 at rows 64:128 ----
            acc2 = ps_acc.tile([128, 512], F32, tag="acc", name=f"acc{s}h2")
            G2 = 4
            for g in range(n // G2):
                for kc in range(g * G2, (g + 1) * G2, 2):
                    emit_scores_one(kq2, q2k2, 0, 64, kc, s, off(kc), exps, (2, kc))
                    emit_scores_one(
                        q2k2, kq2, 64, 128, kc + 1, s, off(kc + 1), exps, (2, kc + 1)
                    )
                if g > 0:
                    for kc in range((g - 1) * G2, g * G2):
                        emit_attv_one(2, acc2, kc, s, n, off(kc), exps, (2, kc))
                pop_filler()
                pop_filler()
            for kc in range(n - G2, n):
                emit_attv_one(2, acc2, kc, s, n, off(kc), exps, (2, kc))
            emit_norm(s, 2, acc2)

            # next strip's prep runs as one contiguous block (its outputs are
            # read by the very next scores, so it cannot be deferred); this
            # strip's projection is queued as filler for the next strip.
            if s + 1 < NS:
                emit_prep(s + 1)
            queue_proj(s)

        drain_all()

    nc.compile()
    return nc


def _shard_inputs(x, w_qkv, w_proj):
    in_maps = []
    for c in range(8):
        b, g = c // 4, c % 4
        h0 = 3 * g
        q = slice(h0 * D, (h0 + 2) * D)
        k = slice(E + h0 * D, E + (h0 + 2) * D)
        wqk = np.concatenate(
            [
                w_qkv[:, k],  # k_h0 | k_h1
                w_qkv[:, q],  # q_h0 | q_h1
                w_qkv[:, E + (h0 + 2) * D : E + (h0 + 3) * D],  # k_h2
                w_qkv[:, (h0 + 2) * D : (h0 + 3) * D],  # q_h2
            ],
            axis=1,
        )
        wv = w_qkv[:, 2 * E + h0 * D : 2 * E + (h0 + 3) * D]
        wp = w_proj[h0 * D : (h0 + 3) * D, :]
        in_maps.append(
            {
                "xt": np.ascontiguousarray(x[b].T),
                "wqk": np.ascontiguousarray(wqk),
                "wv": np.ascontiguousarray(wv),
                "wp": np.ascontiguousarray(wp),
            }
        )
    return in_maps


def kernel(x, w_qkv, w_proj):
    x = np.asarray(x, dtype=np.float32)
    w_qkv = np.asarray(w_qkv, dtype=np.float32)
    w_proj = np.asarray(w_proj, dtype=np.float32)

    if "nc" not in _CACHED:
        _CACHED["nc"] = build_nc()
    nc = _CACHED["nc"]

    in_maps = _shard_inputs(x, w_qkv, w_proj)
    trace = bool(int(os.environ.get("KERNEL_TRACE", "0")))
    res = run_bass_kernel_spmd(
        nc, in_maps, core_ids=list(range(8)), trace=trace
    )
    _CACHED["last_results"] = res

    y = np.zeros((2, T, E), dtype=np.float32)
    for c in range(8):
        y[c // 4] += res.results[c]["y"]
    return y
